# revision 36
# baseline (speedup 1.0000x reference)
"""FK velocity loss kernel for Trainium2 (8 NeuronCores, SPMD data parallel).

Math notes (derived from the reference loss):
  * Each 72-float sample holds 8 joints x 3x3 "m".  The 4x4 joint matrix is
    M = [[cross(c0,c1), c0, c1, c2], [0,0,0,1]] with ck = column k of m.
  * FK translation of a 4-joint chain:  z = R0 (R1 (R2 t3 + t2) + t1) + t0
    with Rj = [xj, c0j, c1j], tj = c2j, xj = cross(c0j, c1j).
    Right-association avoids all 3x3 @ 3x3 products and never needs the
    cross product of the depth-3 joints.
  * vel_loss == pos_loss exactly:
    (out_fk - prev) - (gt_fk - prev) = out_fk - gt_fk.  Therefore
    gt_prev_pose does not influence either loss value and is never read.

Engine split per (tile, tensor):
  * DVE: only the multiplies (cross products' 2 muls/sub stay on DVE; the
    chain's 3 term-products per step).
  * TensorE: all chain additions, as bit-exact identity matmuls accumulating
    into PSUM (fp32 x 1.0 through the PE array is exact; verified on HW).
  * ScalarE: copies final z from PSUM to SBUF and issues ring-B DMAs.
  * GpSimd is useless here: its SBUF port is shared with the DVE's second
    read port, so concurrent gpsimd tensor ops serialize against DVE.

Device computes the per-sample FK translations z for output_pose/gt_pose and
ships them to the host; the host does the (order-independent) mean of squared
differences in float64.
"""

import numpy as np

import concourse.bass as bass
import concourse.bacc as bacc
import concourse.tile as tile
from concourse import mybir
from concourse.bass_utils import run_bass_kernel_spmd

B = 262144
N_CORES = 8
PER_CORE = B // N_CORES        # 32768 samples per core
P = 128                        # SBUF partitions
COLS = PER_CORE // P           # 256 samples per partition per core
F32 = mybir.dt.float32

# Tile plan: S values; sum must equal COLS.  Each tile covers 128*S samples.
DEFAULT_PLAN = (64, 64, 64, 64)


def _lead(ap, step, count):
    """Insert a free dim (step in elements) right after the partition dim."""
    return bass.AP(
        tensor=ap.tensor,
        offset=ap.offset,
        ap=[ap.ap[0], [step, count]] + list(ap.ap[1:]),
    )


def build_nc(cols=COLS, plan=DEFAULT_PLAN, loop=None, no_dma=False,
             dma_only=False, use_pe="dve_t", m_bufs=4, split_loads="col", fat=False,
             t_bufs=3, x_bufs=3, s_bufs=3):
    assert sum(plan) == cols
    per_core = cols * P

    nc = bacc.Bacc()
    src_out = nc.declare_dram_parameter("output_pose", [per_core, 72], F32, isOutput=False)
    src_gt = nc.declare_dram_parameter("gt_pose", [per_core, 72], F32, isOutput=False)
    ident_in = nc.declare_dram_parameter("ident", [P, P], F32, isOutput=False)
    z_out = nc.declare_dram_parameter("z_out", [P, 6 * cols], F32, isOutput=True)
    z_gt = nc.declare_dram_parameter("z_gt", [P, 6 * cols], F32, isOutput=True)

    with tile.TileContext(nc) as tc:
        import contextlib

        loop_ctx = tc.For_i(0, loop, 1) if loop else contextlib.nullcontext()
        with (
            loop_ctx,
            tc.tile_pool(name="singles", bufs=1) as singles,
            tc.tile_pool(name="m_pool", bufs=m_bufs) as mpool,
            tc.tile_pool(name="x_pool", bufs=x_bufs) as xpool,
            tc.tile_pool(name="small", bufs=s_bufs) as spool,
            tc.tile_pool(name="term", bufs=t_bufs) as tpool,
            tc.tile_pool(name="psum", bufs=6, space="PSUM") as ppool,
            tc.tile_pool(name="z_io", bufs=8) as z_io,
        ):
            if use_pe:
                ident = singles.tile([P, P], F32)
                nc.sync.dma_start(out=ident[:], in_=ident_in[:])
            eng = nc.vector
            m_shared = None
            if no_dma:
                # compute-only benchmarking: all tiles read one memset tile
                shape = [P, 2, max(plan), 2, 4, 9] if fat else [P, max(plan), 2, 4, 9]
                m_shared = singles.tile(shape, F32)
                nc.vector.memset(m_shared[:], 1.0)
            col_base = 0
            if fat:
                # both pose tensors batched into every instruction
                for S in plan:
                    row0 = col_base * P
                    if no_dma:
                        m = m_shared
                    else:
                        m = mpool.tile([P, 2, S, 2, 4, 9], F32, tag="m")
                    if not no_dma:
                        for a, (src, ring) in enumerate(
                            ((src_out, nc.sync), (src_gt, nc.scalar))
                        ):
                            ring.dma_start(
                                out=m[:, a].rearrange("p s c d k -> p (s c d k)"),
                                in_=src[row0 : row0 + P * S, :].rearrange(
                                    "(p s) f -> p (s f)", p=P
                                ),
                            )
                    if dma_only:
                        col_base += S
                        continue

                    # cross products, both tensors per instruction
                    x = xpool.tile([P, 3, 2, S, 2, 3], F32, tag="x")
                    t12 = spool.tile([P, 2, S, 2, 3], F32, tag="t12", bufs=1)
                    for r in range(3):
                        r1, r2 = (r + 1) % 3, (r + 2) % 3
                        eng.tensor_mul(
                            x[:, r],
                            m[:, :, :, :, 0:3, 3 * r1],
                            m[:, :, :, :, 0:3, 3 * r2 + 1],
                        )
                        eng.tensor_mul(
                            t12[:],
                            m[:, :, :, :, 0:3, 3 * r2],
                            m[:, :, :, :, 0:3, 3 * r1 + 1],
                        )
                        eng.tensor_sub(x[:, r], x[:, r], t12[:])

                    va = spool.tile([P, 3, 2, S, 2], F32, tag="va", bufs=1)
                    vb = spool.tile([P, 3, 2, S, 2], F32, tag="vb", bufs=1)
                    tm = spool.tile([P, 3, 2, S, 2], F32, tag="tm", bufs=1)
                    z = z_io.tile([P, 3, 2, S, 2], F32, tag="z", bufs=4)

                    def fcolr(d, c):
                        return _lead(m[:, :, :, :, d, c], 3, 3)

                    def fstep(d, vin, vout):
                        eng.tensor_mul(
                            vout[:], x[:, :, :, :, :, d], _lead(vin[0], 0, 3)
                        )
                        eng.tensor_mul(tm[:], fcolr(d, 0), _lead(vin[1], 0, 3))
                        eng.tensor_add(vout[:], vout[:], tm[:])
                        eng.tensor_mul(tm[:], fcolr(d, 1), _lead(vin[2], 0, 3))
                        eng.tensor_add(vout[:], vout[:], tm[:])
                        eng.tensor_add(vout[:], vout[:], fcolr(d, 2))

                    t3 = [m[:, :, :, :, 3, 3 * k + 2] for k in range(3)]
                    fstep(2, t3, va)
                    fstep(1, [va[:, k] for k in range(3)], vb)
                    fstep(0, [vb[:, k] for k in range(3)], z)

                    for a, zdst in enumerate((z_out, z_gt)):
                        nc.gpsimd.dma_start(
                            out=zdst[:, 6 * col_base : 6 * (col_base + S)].rearrange(
                                "p (r s c) -> p r s c", r=3, s=S
                            ),
                            in_=z[:, :, a],
                        )
                    col_base += S
                plan = ()  # skip the per-tensor path below
            for S in plan:
                row0 = col_base * P
                for ti, (src, zdst) in enumerate(((src_out, z_out), (src_gt, z_gt))):
                    dma_eng = nc.sync if ti == 0 else nc.scalar
                    # m layout per partition: S samples x (chain 2, depth 4, k 9)
                    if no_dma:
                        m = m_shared
                    else:
                        m = mpool.tile([P, S, 2, 4, 9], F32, tag="m")
                    if not no_dma:
                        flat = m[:].rearrange("p s c d k -> p (s c d k)")
                        srcv = src[row0 : row0 + P * S, :].rearrange(
                            "(p s) f -> p (s f)", p=P
                        )
                        if split_loads == "col":
                            # split each load across both HWDGE rings (SP + ACT)
                            h = (S // 2) * 72
                            nc.sync.dma_start(out=flat[:, :h], in_=srcv[:, :h])
                            nc.scalar.dma_start(out=flat[:, h:], in_=srcv[:, h:])
                        elif split_loads == "col3":
                            # thirds across SP + ACT HWDGE and gpsimd SWDGE
                            h1 = (S // 3) * 72
                            h2 = (2 * S // 3) * 72
                            nc.sync.dma_start(out=flat[:, :h1], in_=srcv[:, :h1])
                            nc.scalar.dma_start(out=flat[:, h1:h2], in_=srcv[:, h1:h2])
                            nc.gpsimd.dma_start(out=flat[:, h2:], in_=srcv[:, h2:])
                        elif split_loads == "part":
                            # partition split: ring A drives SDMA engines 0-7,
                            # ring B engines 8-15 (engine k owns partitions 8k..)
                            nc.sync.dma_start(out=flat[0:64, :], in_=srcv[0:64, :])
                            nc.scalar.dma_start(out=flat[64:128, :], in_=srcv[64:128, :])
                        else:
                            dma_eng.dma_start(out=flat[:], in_=srcv[:])
                    if dma_only:
                        continue

                    # cross products for depths 0..2 of both chains (DVE)
                    # x[r] = m[3*r1]*m[3*r2+1] - m[3*r2]*m[3*r1+1], rk=(r+k)%3
                    x = xpool.tile([P, 3, S, 2, 3], F32, tag="x")
                    tmp6 = spool.tile([P, S, 2, 3], F32, tag="t6")
                    for r in range(3):
                        r1, r2 = (r + 1) % 3, (r + 2) % 3
                        eng.tensor_mul(
                            x[:, r], m[:, :, :, 0:3, 3 * r1], m[:, :, :, 0:3, 3 * r2 + 1]
                        )
                        eng.tensor_mul(
                            tmp6[:], m[:, :, :, 0:3, 3 * r2], m[:, :, :, 0:3, 3 * r1 + 1]
                        )
                        eng.tensor_sub(x[:, r], x[:, r], tmp6[:])

                    def colr(d, c):
                        # [P, 3(r), S, 2]: element r of column c of joint depth d
                        return _lead(m[:, :, :, d, c], 3, 3)

                    # chain: v <- R_d v + t_d for d = 2, 1, 0 (v init = t3)
                    if use_pe == "dve_t":
                        # PE sums the 3 term products; DVE adds t_d from PSUM
                        # into SBUF (frees the PSUM bank immediately, one bank
                        # in flight per step, no ACT copy for z).
                        def step(d, vin, vout_sbuf):
                            ta = tpool.tile([P, 3, S, 2], F32, tag="ta")
                            tb = tpool.tile([P, 3, S, 2], F32, tag="tb")
                            tc_ = tpool.tile([P, 3, S, 2], F32, tag="tc")
                            vp = ppool.tile([P, 3, S, 2], F32, tag="v")
                            eng.tensor_mul(ta[:], x[:, :, :, :, d], _lead(vin[0], 0, 3))
                            eng.tensor_mul(tb[:], colr(d, 0), _lead(vin[1], 0, 3))
                            eng.tensor_mul(tc_[:], colr(d, 1), _lead(vin[2], 0, 3))
                            nc.tensor.matmul(vp[:], ident[:], ta[:],
                                             start=True, stop=False)
                            nc.tensor.matmul(vp[:], ident[:], tb[:],
                                             start=False, stop=False)
                            nc.tensor.matmul(vp[:], ident[:], tc_[:],
                                             start=False, stop=True)
                            eng.tensor_add(vout_sbuf[:], vp[:], colr(d, 2))

                        va = spool.tile([P, 3, S, 2], F32, tag="va")
                        vb = spool.tile([P, 3, S, 2], F32, tag="vb")
                        z = z_io.tile([P, 3, S, 2], F32, tag="z")
                        t3 = [m[:, :, :, 3, 3 * k + 2] for k in range(3)]
                        step(2, t3, va)
                        step(1, [va[:, k] for k in range(3)], vb)
                        step(0, [vb[:, k] for k in range(3)], z)
                    elif use_pe:
                        def step(d, vin, vout_psum):
                            ta = tpool.tile([P, 3, S, 2], F32, tag="ta")
                            tb = tpool.tile([P, 3, S, 2], F32, tag="tb")
                            tc_ = tpool.tile([P, 3, S, 2], F32, tag="tc")
                            eng.tensor_mul(ta[:], x[:, :, :, :, d], _lead(vin[0], 0, 3))
                            eng.tensor_mul(tb[:], colr(d, 0), _lead(vin[1], 0, 3))
                            eng.tensor_mul(tc_[:], colr(d, 1), _lead(vin[2], 0, 3))
                            nc.tensor.matmul(vout_psum[:], ident[:], ta[:],
                                             start=True, stop=False)
                            nc.tensor.matmul(vout_psum[:], ident[:], tb[:],
                                             start=False, stop=False)
                            nc.tensor.matmul(vout_psum[:], ident[:], tc_[:],
                                             start=False, stop=False)
                            nc.tensor.matmul(vout_psum[:], ident[:], colr(d, 2),
                                             start=False, stop=True)

                        va = ppool.tile([P, 3, S, 2], F32, tag="v")
                        vb = ppool.tile([P, 3, S, 2], F32, tag="v")
                        vz = ppool.tile([P, 3, S, 2], F32, tag="v")
                        t3 = [m[:, :, :, 3, 3 * k + 2] for k in range(3)]
                        step(2, t3, va)
                        step(1, [va[:, k] for k in range(3)], vb)
                        step(0, [vb[:, k] for k in range(3)], vz)
                        z = z_io.tile([P, 3, S, 2], F32, tag="z")
                        nc.scalar.copy(z[:], vz[:])
                    else:
                        va = spool.tile([P, 3, S, 2], F32, tag="va")
                        vb = spool.tile([P, 3, S, 2], F32, tag="vb")
                        tmp = spool.tile([P, 3, S, 2], F32, tag="tm")
                        z = z_io.tile([P, 3, S, 2], F32, tag="z")

                        def step(d, vin, vout):
                            eng.tensor_mul(vout[:], x[:, :, :, :, d], _lead(vin[0], 0, 3))
                            eng.tensor_mul(tmp[:], colr(d, 0), _lead(vin[1], 0, 3))
                            eng.tensor_add(vout[:], vout[:], tmp[:])
                            eng.tensor_mul(tmp[:], colr(d, 1), _lead(vin[2], 0, 3))
                            eng.tensor_add(vout[:], vout[:], tmp[:])
                            eng.tensor_add(vout[:], vout[:], colr(d, 2))

                        t3 = [m[:, :, :, 3, 3 * k + 2] for k in range(3)]
                        step(2, t3, va)
                        step(1, [va[:, k] for k in range(3)], vb)
                        step(0, [vb[:, k] for k in range(3)], z)

                    # z stores go out on the gpsimd SWDGE ring: a store queued
                    # on a HWDGE ring waits for DVE and head-of-line-blocks
                    # every input load queued behind it on that ring.
                    nc.gpsimd.dma_start(
                        out=zdst[:, 6 * col_base : 6 * (col_base + S)],
                        in_=z[:].rearrange("p r s c -> p (r s c)"),
                    )
                col_base += S
    nc.finalize()
    return nc


_NC_CACHE = {}


def _get_nc(cols=COLS, plan=DEFAULT_PLAN):
    key = (cols, plan)
    if key not in _NC_CACHE:
        _NC_CACHE[key] = build_nc(cols, plan)
    return _NC_CACHE[key]


def make_in_maps(output_pose, gt_pose):
    op = np.ascontiguousarray(output_pose, dtype=np.float32)
    gt = np.ascontiguousarray(gt_pose, dtype=np.float32)
    ident = np.eye(P, dtype=np.float32)
    return [
        {
            "output_pose": op[c * PER_CORE : (c + 1) * PER_CORE],
            "gt_pose": gt[c * PER_CORE : (c + 1) * PER_CORE],
            "ident": ident,
        }
        for c in range(N_CORES)
    ]


def run_device(output_pose, gt_pose, plan=DEFAULT_PLAN, trace=False):
    """Run the SPMD kernel; returns (results_list, BassKernelResults)."""
    nc = _get_nc(COLS, plan)
    in_maps = make_in_maps(output_pose, gt_pose)
    res = run_bass_kernel_spmd(nc, in_maps, list(range(N_CORES)), trace=trace)
    return res.results, res


def kernel(output_pose, gt_pose, gt_prev_pose=None, **_ignored):
    results, _ = run_device(output_pose, gt_pose)
    total = 0.0
    for r in results:
        d = r["z_out"].astype(np.float64) - r["z_gt"].astype(np.float64)
        total += float(np.sum(d * d))
    loss = np.float32(total / (B * 6))
    return (loss, loss)
